# revision 14
# baseline (speedup 1.0000x reference)
import numpy as np
import concourse.bass as bass
import concourse.mybir as mybir
import concourse.tile as tile
from concourse.vector_clock import ScopedClock
from concourse.bass_utils import run_bass_kernel_spmd

f32 = mybir.dt.float32
f32r = mybir.dt.float32r
AF = mybir.ActivationFunctionType
ALU = mybir.AluOpType

B, T, D, F, E = 2, 1024, 1024, 2048, 8
N = B * T          # 2048 tokens
NCORES = 8
TB = 1024          # token block
NTB = N // TB      # 2
NDC = D // 128     # 8 contraction chunks for D
NFC = F // 128     # 16 chunks for F
NDCOL = D // 128   # 8 output-column chunks


class _TC(tile.TileContext):
    # This walrus build rejects TPB_CTRL Drain instructions carrying more
    # than one sem wait; split the exit drain into single-wait drains.
    def _drain_and_barrier(self, tick_clock, wait_clock):
        drain_inst = self.nc.sync.drain()
        wait_clock.add_sem_waits(
            drain_inst.ins, ScopedClock({None: tick_clock.global_clock})
        )
        si = drain_inst.ins.sync_info
        if si is not None and si.on_wait and len(si.on_wait) > 1:
            waits = list(si.on_wait)
            si.on_wait = [waits[0]]
            drain_inst.ins.sync_info = si
            for w in waits[1:]:
                d2 = self.nc.sync.drain()
                si2 = d2.ins.sync_info
                if si2 is None:
                    si2 = mybir.SyncInfo(on_wait=[w], on_update=[])
                else:
                    si2.on_wait = [w]
                d2.ins.sync_info = si2
        self.nc.all_engine_barrier()
        assert self.sems is not None
        popped = self.nc._tile_sem_poison_stack.pop()
        assert popped is self._sem_poison
        self.nc.clear_and_free_semaphores(list(self.sems.allocated().values()))
        self.nc.all_engine_barrier()


def _split_multi_waits(nc):
    # This walrus build accepts at most one sem wait per instruction; move
    # extra waits onto pure EventSemaphore waiters inserted just before.
    for fn in nc.m.functions:
        for b in fn.blocks:
            new = []
            changed = False
            for inst in b.instructions:
                si = inst.sync_info
                if si is not None and si.on_wait and len(si.on_wait) > 1:
                    waits = list(si.on_wait)
                    for k, w in enumerate(waits[:-1]):
                        new.append(mybir.InstEventSemaphore(
                            name=f"{inst.name}_xw{k}",
                            engine=inst.engine,
                            sync_info=mybir.SyncInfo(on_wait=[w], on_update=[]),
                        ))
                    si.on_wait = [waits[-1]]
                    inst.sync_info = si
                    changed = True
                new.append(inst)
            if changed:
                b.instructions = new


def build_nc(reps: int = 1):
    nc = bass.Bass()
    xT = nc.declare_dram_parameter("xT", [128, NDC * N], f32r, isOutput=False)
    wg = nc.declare_dram_parameter("wg", [128, NDC * E], f32r, isOutput=False)
    win = nc.declare_dram_parameter("win", [128, NFC * NDC * 128], f32r, isOutput=False)
    wsc = nc.declare_dram_parameter("wsc", [128, NFC * NDC * 128], f32r, isOutput=False)
    wout = nc.declare_dram_parameter("wout", [128, NDCOL * NFC * 128], f32r, isOutput=False)
    sel = nc.declare_dram_parameter("sel", [8, 128], f32, isOutput=False)
    eye = nc.declare_dram_parameter("eye", [128, 128], f32, isOutput=False)
    outT = nc.declare_dram_parameter("outT", [D, N], f32, isOutput=True)

    with _TC(nc) as tc:
        with tc.tile_pool(name="persist", bufs=1) as pp, \
             tc.tile_pool(name="stream", bufs=2) as sp, \
             tc.tile_pool(name="small", bufs=2) as mp, \
             tc.tile_pool(name="psum", bufs=1, space="PSUM") as qp:

            def body():
                xt = pp.tile([128, NDC * N], f32r, tag="xt")
                nc.sync.dma_start(xt, xT[:, :])
                wgt = pp.tile([128, NDC * E], f32r, tag="wgt")
                nc.sync.dma_start(wgt, wg[:, :])
                eyet = pp.tile([128, 128], f32, tag="eyet")
                nc.sync.dma_start(eyet, eye[:, :])
                selt = pp.tile([8, 128], f32, tag="selt")
                nc.sync.dma_start(selt, sel[:, :])

                # ---- router: logitsT [8, N] = Wg^T @ x^T ----
                wb = pp.tile([128, N], f32, tag="wb")  # combine weight bcast
                for c4 in range(N // 512):
                    lg_ps = qp.tile([8, 512], f32, tag="mm1", bufs=3, name="lg_ps")
                    for dc in range(NDC):
                        nc.tensor.matmul(
                            lg_ps,
                            wgt[:, dc * E:(dc + 1) * E],
                            xt[:, dc * N + c4 * 512: dc * N + (c4 + 1) * 512],
                            start=(dc == 0), stop=(dc == NDC - 1),
                        )
                    lg_s = mp.tile([8, 512], f32, tag="lg_s")
                    nc.scalar.copy(lg_s, lg_ps)
                    for cc in range(4):  # 128-token chunks within the 512
                        c = c4 * 4 + cc
                        lgc_ps = qp.tile([128, 8], f32, tag="mm1", bufs=3, name="lgc_ps")
                        nc.tensor.transpose(lgc_ps, lg_s[:, cc * 128:(cc + 1) * 128], eyet[0:8, 0:8])
                        lgc = mp.tile([128, 8], f32, tag="lgc")
                        nc.scalar.copy(lgc, lgc_ps)
                        m8 = mp.tile([128, 8], f32, tag="m8")
                        nc.vector.max(m8, lgc)
                        diff = mp.tile([128, 1], f32, tag="diff")
                        nc.vector.tensor_sub(diff, m8[:, 0:1], m8[:, 1:2])
                        w1 = mp.tile([128, 1], f32, tag="w1")
                        nc.scalar.activation(w1, diff, AF.Sigmoid)
                        w2 = mp.tile([128, 1], f32, tag="w2")
                        nc.scalar.activation(w2, w1, AF.Identity, bias=1.0, scale=-1.0)
                        eq1 = mp.tile([128, 8], f32, tag="eq1")
                        nc.vector.tensor_tensor(eq1, lgc, m8[:, 0:1].to_broadcast([128, 8]), ALU.is_equal)
                        eq2 = mp.tile([128, 8], f32, tag="eq2")
                        nc.vector.tensor_tensor(eq2, lgc, m8[:, 1:2].to_broadcast([128, 8]), ALU.is_equal)
                        nc.vector.tensor_scalar(eq1, eq1, w1, None, op0=ALU.mult)
                        nc.vector.tensor_scalar(eq2, eq2, w2, None, op0=ALU.mult)
                        wf = mp.tile([128, 8], f32, tag="wf")
                        nc.vector.tensor_add(wf, eq1, eq2)
                        # transpose [128,8] -> [8,128], then broadcast row e to
                        # all 128 partitions via one-hot matmul
                        wt_ps = qp.tile([8, 128], f32, tag="mm1", bufs=3, name="wt_ps")
                        nc.tensor.transpose(wt_ps, wf, eyet)
                        wt_s = mp.tile([8, 128], f32, tag="wt_s")
                        nc.scalar.copy(wt_s, wt_ps)
                        wb_ps = qp.tile([128, 128], f32, tag="mm1", bufs=3, name="wb_ps")
                        nc.tensor.matmul(wb_ps, selt, wt_s, start=True, stop=True)
                        nc.scalar.copy(wb[:, c * 128:(c + 1) * 128], wb_ps)

                # ---- expert MLP (dense over all tokens) ----
                inn = pp.tile([128, NFC * TB], f32r, tag="inn")
                for tb in range(NTB):
                    t0 = tb * TB
                    for fc in range(NFC):
                        wi = sp.tile([128, NDC * 128], f32r, tag="wi")
                        nc.sync.dma_start(wi, win[:, fc * 1024:(fc + 1) * 1024])
                        ws = sp.tile([128, NDC * 128], f32r, tag="ws")
                        nc.gpsimd.dma_start(ws, wsc[:, fc * 1024:(fc + 1) * 1024])
                        hp = qp.tile([128, TB], f32, tag="mm1", bufs=3, name="hp")
                        for half in range(TB // 512):
                            for dc in range(NDC):
                                nc.tensor.matmul(
                                    hp[:, half * 512: half * 512 + 512],
                                    wi[:, dc * 128:(dc + 1) * 128],
                                    xt[:, dc * N + t0 + half * 512: dc * N + t0 + half * 512 + 512],
                                    start=(dc == 0), stop=(dc == NDC - 1),
                                )
                        spp = qp.tile([128, TB], f32, tag="mm1", bufs=3, name="spp")
                        for half in range(TB // 512):
                            for dc in range(NDC):
                                nc.tensor.matmul(
                                    spp[:, half * 512: half * 512 + 512],
                                    ws[:, dc * 128:(dc + 1) * 128],
                                    xt[:, dc * N + t0 + half * 512: dc * N + t0 + half * 512 + 512],
                                    start=(dc == 0), stop=(dc == NDC - 1),
                                )
                        g = sp.tile([128, TB], f32, tag="g")
                        nc.scalar.activation(g, hp, AF.Gelu)
                        nc.vector.tensor_mul(inn[:, fc * TB:(fc + 1) * TB], g, spp)
                    for dcol in range(NDCOL):
                        wo = sp.tile([128, NFC * 128], f32r, tag="wo")
                        nc.gpsimd.dma_start(wo, wout[:, dcol * 2048:(dcol + 1) * 2048])
                        op_ = qp.tile([128, TB], f32, tag="mm2", bufs=1, name="op_")
                        for half in range(TB // 512):
                            for fc in range(NFC):
                                nc.tensor.matmul(
                                    op_[:, half * 512: half * 512 + 512],
                                    wo[:, fc * 128:(fc + 1) * 128],
                                    inn[:, fc * TB + half * 512: fc * TB + half * 512 + 512],
                                    start=(fc == 0), stop=(fc == NFC - 1),
                                )
                        ot = sp.tile([128, TB], f32, tag="ot")
                        nc.vector.tensor_mul(ot, op_, wb[:, t0:t0 + TB])
                        nc.sync.dma_start(outT[dcol * 128:(dcol + 1) * 128, t0:t0 + TB], ot)

            # For_i is unusable here: this walrus build rejects the loop's
            # InstIncSwdgeSem sem-reset (ISA wrong length), so unroll.
            for _ in range(reps):
                body()
    _split_multi_waits(nc)
    return nc


def _prep_core_inputs(states, Wg, W_in, W_scale, W_out):
    x = np.ascontiguousarray(states.reshape(N, D), dtype=np.float32)
    xT_t = np.ascontiguousarray(
        x.T.reshape(NDC, 128, N).transpose(1, 0, 2).reshape(128, NDC * N)
    )
    wg_t = np.ascontiguousarray(
        Wg.reshape(NDC, 128, E).transpose(1, 0, 2).reshape(128, NDC * E)
    )
    eye = np.eye(128, dtype=np.float32)
    in_maps = []
    for e in range(NCORES):
        win_t = np.ascontiguousarray(
            W_in[e].reshape(NDC, 128, NFC, 128).transpose(1, 2, 0, 3).reshape(128, NFC * NDC * 128)
        )
        wsc_t = np.ascontiguousarray(
            W_scale[e].reshape(NDC, 128, NFC, 128).transpose(1, 2, 0, 3).reshape(128, NFC * NDC * 128)
        )
        wout_t = np.ascontiguousarray(
            W_out[e].reshape(NFC, 128, NDCOL, 128).transpose(1, 2, 0, 3).reshape(128, NDCOL * NFC * 128)
        )
        sel_e = np.zeros((8, 128), dtype=np.float32)
        sel_e[e, :] = 1.0
        in_maps.append({
            "xT": xT_t, "wg": wg_t, "win": win_t, "wsc": wsc_t,
            "wout": wout_t, "sel": sel_e, "eye": eye,
        })
    return in_maps


_CACHED_NC = None


def kernel(**inputs) -> np.ndarray:
    global _CACHED_NC
    states = np.asarray(inputs["states"], dtype=np.float32)
    Wg = np.asarray(inputs["Wg"], dtype=np.float32)
    W_in = np.asarray(inputs["W_in"], dtype=np.float32)
    W_scale = np.asarray(inputs["W_scale"], dtype=np.float32)
    W_out = np.asarray(inputs["W_out"], dtype=np.float32)

    in_maps = _prep_core_inputs(states, Wg, W_in, W_scale, W_out)
    if _CACHED_NC is None:
        _CACHED_NC = build_nc(reps=1)
    res = run_bass_kernel_spmd(_CACHED_NC, in_maps, list(range(NCORES)))
    total = np.zeros((D, N), dtype=np.float32)
    for i in range(NCORES):
        total += res.results[i]["outT"]
    return np.ascontiguousarray(total.T).reshape(B, T, D)


# revision 16
# speedup vs baseline: 1.7058x; 1.7058x over previous
import numpy as np
import concourse.bass as bass
import concourse.mybir as mybir
import concourse.tile as tile
from concourse.vector_clock import ScopedClock
from concourse.bass_utils import run_bass_kernel_spmd

f32 = mybir.dt.float32
f32r = mybir.dt.float32r
AF = mybir.ActivationFunctionType
ALU = mybir.AluOpType

B, T, D, F, E = 2, 1024, 1024, 2048, 8
N = B * T          # 2048 tokens
NCORES = 8
C = 1024           # per-expert token capacity (mean load = 512, ~26 sigma margin)
NDC = D // 128     # 8 contraction chunks for D
NFC = F // 128     # 16 chunks for F
NDCOL = D // 128   # 8 output-column chunks
NH = C // 512      # 2 free-dim halves
FCG = 2            # fc per weight-stream DMA
DCG = 2            # dcol per weight-stream DMA

# Kept for test.py compatibility
TB = C
NTB = N // TB


class _TC(tile.TileContext):
    # This walrus build rejects TPB_CTRL Drain instructions carrying more
    # than one sem wait; split the exit drain into single-wait drains.
    def _drain_and_barrier(self, tick_clock, wait_clock):
        drain_inst = self.nc.sync.drain()
        wait_clock.add_sem_waits(
            drain_inst.ins, ScopedClock({None: tick_clock.global_clock})
        )
        si = drain_inst.ins.sync_info
        if si is not None and si.on_wait and len(si.on_wait) > 1:
            waits = list(si.on_wait)
            si.on_wait = [waits[0]]
            drain_inst.ins.sync_info = si
            for w in waits[1:]:
                d2 = self.nc.sync.drain()
                si2 = d2.ins.sync_info
                if si2 is None:
                    si2 = mybir.SyncInfo(on_wait=[w], on_update=[])
                else:
                    si2.on_wait = [w]
                d2.ins.sync_info = si2
        self.nc.all_engine_barrier()
        assert self.sems is not None
        popped = self.nc._tile_sem_poison_stack.pop()
        assert popped is self._sem_poison
        self.nc.clear_and_free_semaphores(list(self.sems.allocated().values()))
        self.nc.all_engine_barrier()


def _split_multi_waits(nc):
    # This walrus build accepts at most one sem wait per instruction; move
    # extra waits onto pure EventSemaphore waiters inserted just before.
    for fn in nc.m.functions:
        for b in fn.blocks:
            new = []
            changed = False
            for inst in b.instructions:
                si = inst.sync_info
                if si is not None and si.on_wait and len(si.on_wait) > 1:
                    waits = list(si.on_wait)
                    for k, w in enumerate(waits[:-1]):
                        new.append(mybir.InstEventSemaphore(
                            name=f"{inst.name}_xw{k}",
                            engine=inst.engine,
                            sync_info=mybir.SyncInfo(on_wait=[w], on_update=[]),
                        ))
                    si.on_wait = [waits[-1]]
                    inst.sync_info = si
                    changed = True
                new.append(inst)
            if changed:
                b.instructions = new


def build_nc(reps: int = 1):
    nc = bass.Bass()
    xg = nc.declare_dram_parameter("xg", [128, NDC * C], f32r, isOutput=False)
    win = nc.declare_dram_parameter("win", [128, NFC * NDC * 128], f32r, isOutput=False)
    wsc = nc.declare_dram_parameter("wsc", [128, NFC * NDC * 128], f32r, isOutput=False)
    wout = nc.declare_dram_parameter("wout", [128, NDCOL * NFC * 128], f32r, isOutput=False)
    yT = nc.declare_dram_parameter("yT", [128, NDCOL * C], f32, isOutput=True)

    with _TC(nc) as tc:
        with tc.tile_pool(name="persist", bufs=1) as pp, \
             tc.tile_pool(name="stream", bufs=2) as sp, \
             tc.tile_pool(name="small", bufs=2) as mp, \
             tc.tile_pool(name="psum", bufs=1, space="PSUM") as qp:

            def body():
                xt = pp.tile([128, NDC * C], f32r, tag="xt")
                nc.sync.dma_start(xt, xg[:, :])
                inner = pp.tile([128, NFC * C], f32r, tag="inner")

                # ---- mm1: h = W_in^T x, s = W_sc^T x; inner = gelu(h)*s ----
                for g4 in range(NFC // FCG):
                    wi_t = sp.tile([128, FCG * NDC * 128], f32r, tag="wi")
                    ws_t = sp.tile([128, FCG * NDC * 128], f32r, tag="ws")
                    nc.sync.dma_start(
                        wi_t, win[:, g4 * FCG * NDC * 128:(g4 + 1) * FCG * NDC * 128])
                    nc.gpsimd.dma_start(
                        ws_t, wsc[:, g4 * FCG * NDC * 128:(g4 + 1) * FCG * NDC * 128])
                    for fci in range(FCG):
                        fc = g4 * FCG + fci
                        for half in range(NH):
                            hp = qp.tile([128, 512], f32, tag="ps", bufs=4, name="hp")
                            for dc in range(NDC):
                                nc.tensor.matmul(
                                    hp,
                                    wi_t[:, (fci * NDC + dc) * 128:(fci * NDC + dc) * 128 + 128],
                                    xt[:, dc * C + half * 512: dc * C + half * 512 + 512],
                                    start=(dc == 0), stop=(dc == NDC - 1),
                                )
                            sps = qp.tile([128, 512], f32, tag="ps", bufs=4, name="sps")
                            for dc in range(NDC):
                                nc.tensor.matmul(
                                    sps,
                                    ws_t[:, (fci * NDC + dc) * 128:(fci * NDC + dc) * 128 + 128],
                                    xt[:, dc * C + half * 512: dc * C + half * 512 + 512],
                                    start=(dc == 0), stop=(dc == NDC - 1),
                                )
                            g = mp.tile([128, 512], f32, tag="g")
                            nc.scalar.activation(g, hp, AF.Gelu)
                            nc.vector.tensor_mul(
                                inner[:, fc * C + half * 512: fc * C + half * 512 + 512],
                                g, sps)

                # ---- mm2: y = W_out^T inner ----
                for g2 in range(NDCOL // DCG):
                    wo_t = sp.tile([128, DCG * NFC * 128], f32r, tag="wo")
                    nc.sync.dma_start(
                        wo_t, wout[:, g2 * DCG * NFC * 128:(g2 + 1) * DCG * NFC * 128])
                    for dci in range(DCG):
                        dcol = g2 * DCG + dci
                        ot = mp.tile([128, C], f32, tag="ot")
                        for half in range(NH):
                            op = qp.tile([128, 512], f32, tag="ps2", bufs=2, name="op")
                            for fc in range(NFC):
                                nc.tensor.matmul(
                                    op,
                                    wo_t[:, (dci * NFC + fc) * 128:(dci * NFC + fc) * 128 + 128],
                                    inner[:, fc * C + half * 512: fc * C + half * 512 + 512],
                                    start=(fc == 0), stop=(fc == NFC - 1),
                                )
                            nc.scalar.copy(ot[:, half * 512:half * 512 + 512], op)
                        nc.sync.dma_start(yT[:, dcol * C:(dcol + 1) * C], ot)

            for _ in range(reps):
                body()
    _split_multi_waits(nc)
    return nc


def _route(states, Wg):
    x = np.ascontiguousarray(states.reshape(N, D), dtype=np.float32)
    logits = x @ np.asarray(Wg, np.float32)                    # [N, E]
    order = np.argsort(-logits, axis=1, kind="stable")[:, :2]  # top-2, desc
    rows = np.arange(N)
    l1 = logits[rows, order[:, 0]]
    l2 = logits[rows, order[:, 1]]
    w1 = 1.0 / (1.0 + np.exp(-(l1 - l2).astype(np.float64)))
    w2 = 1.0 - w1
    return x, order, w1.astype(np.float32), w2.astype(np.float32)


def _prep_core_inputs(states, Wg, W_in, W_scale, W_out):
    x, order, w1, w2 = _route(states, Wg)
    in_maps = []
    idx_list, wtok_list = [], []
    for e in range(E):
        m1 = order[:, 0] == e
        m2 = order[:, 1] == e
        idx = np.where(m1 | m2)[0]
        assert len(idx) <= C, f"expert {e} load {len(idx)} exceeds capacity {C}"
        wtok = np.where(m1, w1, w2)[idx]
        idx_list.append(idx)
        wtok_list.append(wtok)

        xpad = np.zeros((C, D), np.float32)
        xpad[:len(idx)] = x[idx]
        xg_a = np.ascontiguousarray(
            xpad.T.reshape(NDC, 128, C).transpose(1, 0, 2).reshape(128, NDC * C))
        wi = np.ascontiguousarray(
            np.asarray(W_in[e], np.float32)
            .reshape(NDC, 128, NFC, 128).transpose(1, 2, 0, 3)
            .reshape(128, NFC * NDC * 128))
        ws = np.ascontiguousarray(
            np.asarray(W_scale[e], np.float32)
            .reshape(NDC, 128, NFC, 128).transpose(1, 2, 0, 3)
            .reshape(128, NFC * NDC * 128))
        wo = np.ascontiguousarray(
            np.asarray(W_out[e], np.float32)
            .reshape(NFC, 128, NDCOL, 128).transpose(1, 2, 0, 3)
            .reshape(128, NDCOL * NFC * 128))
        in_maps.append({"xg": xg_a, "win": wi, "wsc": ws, "wout": wo})
    return in_maps, idx_list, wtok_list


_CACHED_NC = None


def kernel(**inputs):
    global _CACHED_NC
    states = np.asarray(inputs["states"], np.float32)
    in_maps, idx_list, wtok_list = _prep_core_inputs(
        states,
        np.asarray(inputs["Wg"], np.float32),
        np.asarray(inputs["W_in"], np.float32),
        np.asarray(inputs["W_scale"], np.float32),
        np.asarray(inputs["W_out"], np.float32),
    )
    if _CACHED_NC is None:
        _CACHED_NC = build_nc(1)
    res = run_bass_kernel_spmd(_CACHED_NC, in_maps, list(range(NCORES)))
    out = np.zeros((N, D), np.float32)
    for e in range(E):
        yTe = np.asarray(res.results[e]["yT"])  # [128, NDCOL*C]
        y = yTe.reshape(128, NDCOL, C).transpose(1, 0, 2).reshape(D, C)
        idx = idx_list[e]
        out[idx] += wtok_list[e][:, None] * y[:, :len(idx)].T
    return out.reshape(B, T, D)


# revision 17
# speedup vs baseline: 2.0452x; 1.1989x over previous
import numpy as np
import concourse.bass as bass
import concourse.mybir as mybir
import concourse.tile as tile
from concourse.vector_clock import ScopedClock
from concourse.bass_utils import run_bass_kernel_spmd

f32 = mybir.dt.float32
f32r = mybir.dt.float32r
AF = mybir.ActivationFunctionType
ALU = mybir.AluOpType

B, T, D, F, E = 2, 1024, 1024, 2048, 8
N = B * T          # 2048 tokens
NCORES = 8
C = 1024           # per-expert token capacity (mean load = 512, ~26 sigma margin)
NDC = D // 128     # 8 contraction chunks for D
NFC = F // 128     # 16 chunks for F
NDCOL = D // 128   # 8 output-column chunks
NH = C // 512      # 2 free-dim halves
FCG = 2            # fc per weight-stream DMA
DCG = 2            # dcol per weight-stream DMA

# Kept for test.py compatibility
TB = C
NTB = N // TB


class _TC(tile.TileContext):
    # This walrus build rejects TPB_CTRL Drain instructions carrying more
    # than one sem wait; split the exit drain into single-wait drains.
    def _drain_and_barrier(self, tick_clock, wait_clock):
        drain_inst = self.nc.sync.drain()
        wait_clock.add_sem_waits(
            drain_inst.ins, ScopedClock({None: tick_clock.global_clock})
        )
        si = drain_inst.ins.sync_info
        if si is not None and si.on_wait and len(si.on_wait) > 1:
            waits = list(si.on_wait)
            si.on_wait = [waits[0]]
            drain_inst.ins.sync_info = si
            for w in waits[1:]:
                d2 = self.nc.sync.drain()
                si2 = d2.ins.sync_info
                if si2 is None:
                    si2 = mybir.SyncInfo(on_wait=[w], on_update=[])
                else:
                    si2.on_wait = [w]
                d2.ins.sync_info = si2
        self.nc.all_engine_barrier()
        assert self.sems is not None
        popped = self.nc._tile_sem_poison_stack.pop()
        assert popped is self._sem_poison
        self.nc.clear_and_free_semaphores(list(self.sems.allocated().values()))
        self.nc.all_engine_barrier()


def _split_multi_waits(nc):
    # This walrus build accepts at most one sem wait per instruction; move
    # extra waits onto pure EventSemaphore waiters inserted just before.
    for fn in nc.m.functions:
        for b in fn.blocks:
            new = []
            changed = False
            for inst in b.instructions:
                si = inst.sync_info
                if si is not None and si.on_wait and len(si.on_wait) > 1:
                    waits = list(si.on_wait)
                    for k, w in enumerate(waits[:-1]):
                        new.append(mybir.InstEventSemaphore(
                            name=f"{inst.name}_xw{k}",
                            engine=inst.engine,
                            sync_info=mybir.SyncInfo(on_wait=[w], on_update=[]),
                        ))
                    si.on_wait = [waits[-1]]
                    inst.sync_info = si
                    changed = True
                new.append(inst)
            if changed:
                b.instructions = new


def build_nc(reps: int = 1):
    nc = bass.Bass()
    xg = nc.declare_dram_parameter("xg", [128, NDC * C], f32r, isOutput=False)
    win = nc.declare_dram_parameter("win", [128, NFC * NDC * 128], f32r, isOutput=False)
    wsc = nc.declare_dram_parameter("wsc", [128, NFC * NDC * 128], f32r, isOutput=False)
    wout = nc.declare_dram_parameter("wout", [128, NDCOL * NFC * 128], f32r, isOutput=False)
    yT = nc.declare_dram_parameter("yT", [128, NDCOL * C], f32, isOutput=True)

    with _TC(nc) as tc:
        with tc.tile_pool(name="persist", bufs=1) as pp, \
             tc.tile_pool(name="stream", bufs=2) as sp, \
             tc.tile_pool(name="small", bufs=2) as mp, \
             tc.tile_pool(name="psum", bufs=1, space="PSUM") as qp:

            def body():
                xt = pp.tile([128, NDC * C], f32r, tag="xt")
                nc.sync.dma_start(xt, xg[:, :])
                inner = pp.tile([128, NFC * C], f32r, tag="inner")

                # ---- mm1: h = W_in^T x, s = W_sc^T x; inner = gelu(h)*s ----
                for g4 in range(NFC // FCG):
                    wi_t = sp.tile([128, FCG * NDC * 128], f32r, tag="wi")
                    ws_t = sp.tile([128, FCG * NDC * 128], f32r, tag="ws")
                    nc.sync.dma_start(
                        wi_t, win[:, g4 * FCG * NDC * 128:(g4 + 1) * FCG * NDC * 128])
                    nc.gpsimd.dma_start(
                        ws_t, wsc[:, g4 * FCG * NDC * 128:(g4 + 1) * FCG * NDC * 128])
                    for fci in range(FCG):
                        fc = g4 * FCG + fci
                        for half in range(NH):
                            hp = qp.tile([128, 512], f32, tag="ps", bufs=6, name="hp")
                            for dc in range(NDC):
                                nc.tensor.matmul(
                                    hp,
                                    wi_t[:, (fci * NDC + dc) * 128:(fci * NDC + dc) * 128 + 128],
                                    xt[:, dc * C + half * 512: dc * C + half * 512 + 512],
                                    start=(dc == 0), stop=(dc == NDC - 1),
                                )
                            sps = qp.tile([128, 512], f32, tag="ps", bufs=6, name="sps")
                            for dc in range(NDC):
                                nc.tensor.matmul(
                                    sps,
                                    ws_t[:, (fci * NDC + dc) * 128:(fci * NDC + dc) * 128 + 128],
                                    xt[:, dc * C + half * 512: dc * C + half * 512 + 512],
                                    start=(dc == 0), stop=(dc == NDC - 1),
                                )
                            g = mp.tile([128, 512], f32, tag="g")
                            nc.scalar.activation(g, hp, AF.Gelu)
                            nc.vector.tensor_mul(
                                inner[:, fc * C + half * 512: fc * C + half * 512 + 512],
                                g, sps)

                # ---- mm2: y = W_out^T inner ----
                for g2 in range(NDCOL // DCG):
                    wo_t = sp.tile([128, DCG * NFC * 128], f32r, tag="wo")
                    nc.sync.dma_start(
                        wo_t, wout[:, g2 * DCG * NFC * 128:(g2 + 1) * DCG * NFC * 128])
                    for dci in range(DCG):
                        dcol = g2 * DCG + dci
                        ot = mp.tile([128, C], f32, tag="ot")
                        for half in range(NH):
                            op = qp.tile([128, 512], f32, tag="ps", bufs=6, name="op")
                            for fc in range(NFC):
                                nc.tensor.matmul(
                                    op,
                                    wo_t[:, (dci * NFC + fc) * 128:(dci * NFC + fc) * 128 + 128],
                                    inner[:, fc * C + half * 512: fc * C + half * 512 + 512],
                                    start=(fc == 0), stop=(fc == NFC - 1),
                                )
                            nc.scalar.copy(ot[:, half * 512:half * 512 + 512], op)
                        nc.sync.dma_start(yT[:, dcol * C:(dcol + 1) * C], ot)

            for _ in range(reps):
                body()
    _split_multi_waits(nc)
    return nc


def _route(states, Wg):
    x = np.ascontiguousarray(states.reshape(N, D), dtype=np.float32)
    logits = x @ np.asarray(Wg, np.float32)                    # [N, E]
    order = np.argsort(-logits, axis=1, kind="stable")[:, :2]  # top-2, desc
    rows = np.arange(N)
    l1 = logits[rows, order[:, 0]]
    l2 = logits[rows, order[:, 1]]
    w1 = 1.0 / (1.0 + np.exp(-(l1 - l2).astype(np.float64)))
    w2 = 1.0 - w1
    return x, order, w1.astype(np.float32), w2.astype(np.float32)


def _prep_core_inputs(states, Wg, W_in, W_scale, W_out):
    x, order, w1, w2 = _route(states, Wg)
    in_maps = []
    idx_list, wtok_list = [], []
    for e in range(E):
        m1 = order[:, 0] == e
        m2 = order[:, 1] == e
        idx = np.where(m1 | m2)[0]
        assert len(idx) <= C, f"expert {e} load {len(idx)} exceeds capacity {C}"
        wtok = np.where(m1, w1, w2)[idx]
        idx_list.append(idx)
        wtok_list.append(wtok)

        xpad = np.zeros((C, D), np.float32)
        xpad[:len(idx)] = x[idx]
        xg_a = np.ascontiguousarray(
            xpad.T.reshape(NDC, 128, C).transpose(1, 0, 2).reshape(128, NDC * C))
        wi = np.ascontiguousarray(
            np.asarray(W_in[e], np.float32)
            .reshape(NDC, 128, NFC, 128).transpose(1, 2, 0, 3)
            .reshape(128, NFC * NDC * 128))
        ws = np.ascontiguousarray(
            np.asarray(W_scale[e], np.float32)
            .reshape(NDC, 128, NFC, 128).transpose(1, 2, 0, 3)
            .reshape(128, NFC * NDC * 128))
        wo = np.ascontiguousarray(
            np.asarray(W_out[e], np.float32)
            .reshape(NFC, 128, NDCOL, 128).transpose(1, 2, 0, 3)
            .reshape(128, NDCOL * NFC * 128))
        in_maps.append({"xg": xg_a, "win": wi, "wsc": ws, "wout": wo})
    return in_maps, idx_list, wtok_list


_CACHED_NC = None


def kernel(**inputs):
    global _CACHED_NC
    states = np.asarray(inputs["states"], np.float32)
    in_maps, idx_list, wtok_list = _prep_core_inputs(
        states,
        np.asarray(inputs["Wg"], np.float32),
        np.asarray(inputs["W_in"], np.float32),
        np.asarray(inputs["W_scale"], np.float32),
        np.asarray(inputs["W_out"], np.float32),
    )
    if _CACHED_NC is None:
        _CACHED_NC = build_nc(1)
    res = run_bass_kernel_spmd(_CACHED_NC, in_maps, list(range(NCORES)))
    out = np.zeros((N, D), np.float32)
    for e in range(E):
        yTe = np.asarray(res.results[e]["yT"])  # [128, NDCOL*C]
        y = yTe.reshape(128, NDCOL, C).transpose(1, 0, 2).reshape(D, C)
        idx = idx_list[e]
        out[idx] += wtok_list[e][:, None] * y[:, :len(idx)].T
    return out.reshape(B, T, D)
